# revision 11
# baseline (speedup 1.0000x reference)
"""Trainium2 Bass kernel: 3x3 stride-1 VALID conv (NHWC, HWIO) + bias + ReLU.

Problem shapes:
  x       (32, 112, 112, 64)  f32
  kernels (3, 3, 64, 128)     f32
  biases  (128,)              f32
  out     (32, 110, 110, 128) f32

Strategy:
  * Data-parallel: 4 images per core across 8 NeuronCores (no collectives).
  * Host pre-packs x into a channel/row-parity-major layout
      X[p, rp, b, w]  with p = (h%2)*64 + c,  rp = h//2
    so adjacent image rows sit on opposite halves of the 128 SBUF
    partitions. A 3x3x64 conv then becomes 6 PSUM-accumulated matmuls
    per output row (3 kw shifts x {one K=128 row-pair matmul + one K=64
    single-row matmul}), batching all 4 images into N=440 moving columns.
  * The six K=64 "leftover" matmuls per row pair land on opposite PE
    row-halves (even rows use partitions 0-63, odd rows 64-127), so the
    hardware runs them as three concurrent row-tiled pairs -> 9 effective
    PE slots per row pair = the MAC-floor schedule (4.5 slots/row).
  * Switching the PE between 128-row and 64-row tiling modes drains the
    array (~95ns). a-iterations are processed in pairs ordered
    [A K128s][A singles][B singles][B K128s] so the drain is paid twice
    per TWO row pairs instead of twice per one.
  * Warm-up matmuls are emitted before any input-DMA trigger so the PE
    HAM clock gate (cold = 1.2GHz) releases while input DMAs are still
    in flight; input chunk triggers alternate sync/gpsimd queues so the
    first chunks land ~2us earlier than a single serial trigger queue.
  * fp32r/fp16 matmul dtype: fp16 streams 1 col/cycle with fp32 PSUM
    accumulation and ~3e-4 conv error (vs 2e-2 budget).
  * ScalarE fuses bias+ReLU on the PSUM->SBUF evacuation, writing fp16
    to halve output DMA bytes; host restores NHWC f32.
"""

import numpy as np

import concourse.bass as bass
import concourse.mybir as mybir
from concourse import bacc
from concourse.bass_utils import run_bass_kernel_spmd
from concourse.tile import TileContext

N_CORES = 8
B = 4  # images per core
H = W = 112
C = 64
F = 128
KH = KW = 3
HO = WO = 110
NRP = H // 2  # 56 row pairs per image
A = HO // 2  # 55 output row-parity iterations

F32 = mybir.dt.float32
F16 = mybir.dt.float16
MM_DTYPE = F16
OUT_DTYPE = F16

X_ELEMS = NRP * B * W  # per-partition input elements (25088)
O_ELEMS = A * 2 * B * WO  # per-partition output elements (48400)

_TRACE = False
LAST_RESULT = None
_NC_CACHE = None


def _build_bass():
    nc = bacc.Bacc("TRN2", target_bir_lowering=False, debug=False)
    x_d = nc.dram_tensor("x", [128, X_ELEMS], MM_DTYPE, kind="ExternalInput")
    # weights (9 stacked [128,128] lhsT tiles) + fp32 bias packed as the
    # last two fp16 columns (bitcast back to f32 on device)
    w_d = nc.dram_tensor("w", [128, 9 * F + 2], MM_DTYPE, kind="ExternalInput")
    o_d = nc.dram_tensor("o", [128, O_ELEMS], OUT_DTYPE, kind="ExternalOutput")

    rpw = B * W  # elems per rowpair per partition (448)
    ow = 2 * B * WO  # output elems per a-iteration (880)

    with TileContext(nc) as tc:
        with (
            tc.tile_pool(name="xres", bufs=1) as xpool,
            tc.tile_pool(name="wpool", bufs=1) as wpool,
            tc.tile_pool(name="psum", bufs=8, space="PSUM") as ppool,
            tc.tile_pool(name="opool", bufs=4) as opool,
        ):
            # PE pre-warm FIRST: dummy matmuls on a vector-memset buffer
            # while weights/input DMAs are still being triggered. Releases
            # the PE_HAM clock gate (cold = 1.2GHz) before real work
            # arrives; the scratch PSUM bank is never read. ~11 cold
            # N=440 matmuls x 367ns spans the gap until chunk data lands.
            warm = wpool.tile([128, 440], MM_DTYPE)
            nc.vector.memset(warm[:], 0.0)
            wps = ppool.tile([128, B * WO], F32, tag="ps")
            NWARM = 7
            for j in range(NWARM):
                nc.tensor.matmul(
                    wps[:], warm[:, 0:128], warm[:], start=(j == 0), stop=(j == NWARM - 1)
                )

            wt = wpool.tile([128, 9 * F + 2], MM_DTYPE)
            nc.scalar.dma_start(out=wt[:], in_=w_d[:])
            bt = wt[:, 9 * F : 9 * F + 2].bitcast(F32)

            # Fast-start chunk schedule: small chunks first so the first
            # matmul group can begin ASAP, larger chunks once compute is
            # the slower consumer. Triggers alternate between the sync
            # and gpsimd queues: two engines issue ~680ns DMA triggers in
            # parallel, and neither shares DMAHW lanes with the scalar
            # output-DMA path (head-of-line blocking).
            chunk_rps = [1, 1, 2, 4] + [8] * 6
            assert sum(chunk_rps) == NRP
            trig = [nc.sync, nc.gpsimd]
            rp2view = []  # rowpair -> (view, local index)
            for ch, nrp_ch in enumerate(chunk_rps):
                cht = xpool.tile([128, nrp_ch * rpw], MM_DTYPE, tag=f"xch{ch}")
                s = len(rp2view) * rpw
                trig[ch % 2].dma_start(out=cht[:], in_=x_d[:, s : s + nrp_ch * rpw])
                v = cht[:].rearrange("p (rp b w) -> p rp b w", rp=nrp_ch, b=B, w=W)
                for r in range(nrp_ch):
                    rp2view.append((v, r))

            def xs(lo, hi, rp, kw):
                v, r = rp2view[rp]
                return v[lo:hi, r, :, kw : kw + WO]

            wv = wt[:, 0 : 9 * F].rearrange("p (i f) -> p i f", i=9, f=F)

            def k128s(ps_par0, ps_par1, a, start, stop):
                # six full-array matmuls of one a-iteration:
                # out row 2a: kh=0,1 -> rows 2a,2a+1 (rowpair a, K=128)
                # out row 2a+1: kh=1,2 -> rows 2a+2,2a+3 (rowpair a+1, K=128)
                for kw in range(KW):
                    nc.tensor.matmul(
                        ps_par0, wv[:, kw, :], xs(0, 128, a, kw),
                        start=(start and kw == 0), stop=(stop and kw == KW - 1),
                    )
                for kw in range(KW):
                    nc.tensor.matmul(
                        ps_par1, wv[:, 3 + kw, :], xs(0, 128, a + 1, kw),
                        start=(start and kw == 0), stop=(stop and kw == KW - 1),
                    )

            def singles(ps_par0, ps_par1, a, start, stop):
                # six K=64 matmuls on opposite PE row-halves, emitted
                # interleaved so the hardware runs them as three
                # concurrent row-tiled pairs:
                # out row 2a: kh=2 -> row 2a+2 (low half of rowpair a+1)
                # out row 2a+1: kh=0 -> row 2a+1 (high half of rowpair a)
                for kw in range(KW):
                    nc.tensor.matmul(
                        ps_par0, wv[0:64, 6 + kw, :], xs(0, 64, a + 1, kw),
                        start=(start and kw == 0), stop=(stop and kw == KW - 1),
                    )
                    nc.tensor.matmul(
                        ps_par1, wv[64:128, 6 + kw, :], xs(64, 128, a, kw),
                        start=(start and kw == 0), stop=(stop and kw == KW - 1),
                    )

            def evac(ps, ot, slot, engine="scalar"):
                dst = ot[:, slot * B * WO : (slot + 1) * B * WO]
                if engine == "scalar":
                    nc.scalar.activation(
                        out=dst,
                        in_=ps[:],
                        func=mybir.ActivationFunctionType.Relu,
                        bias=bt,
                    )
                else:
                    # DVE bias+relu: out = max(psum + bias, 0)
                    nc.vector.tensor_scalar(
                        out=dst,
                        in0=ps[:],
                        scalar1=bt,
                        scalar2=0.0,
                        op0=mybir.AluOpType.add,
                        op1=mybir.AluOpType.max,
                    )

            # Process a-iterations in groups of 8 ordered
            # [A..D K][A..H s][E..H K]: the PE pays the two
            # 128row<->64row tiling-mode drains (~95ns each) per EIGHT
            # row pairs. The 8-bank PSUM ring wraps mid-group; each bank
            # is evacuated right after its accumulation stops, well
            # before the ring hands it to a later iteration.
            a = 0
            while a < A:
                n_in_g = min(8, A - a)
                last_group = a + n_in_g >= A
                ot = opool.tile([128, n_in_g * ow], OUT_DTYPE, tag="ot")
                ps = []
                pv = []
                for i in range(n_in_g):
                    p0 = ppool.tile([128, B * WO], F32, tag="ps", name=f"p{i}0")
                    p1 = ppool.tile([128, B * WO], F32, tag="ps", name=f"p{i}1")
                    ps.append((p0, p1))
                    pv.append((
                        p0[:].rearrange("p (b w) -> p b w", b=B),
                        p1[:].rearrange("p (b w) -> p b w", b=B),
                    ))
                nk = (n_in_g + 1) // 2  # iters whose K128s lead
                for i in range(nk):
                    k128s(pv[i][0], pv[i][1], a + i, start=True, stop=False)
                for i in range(n_in_g):
                    stop = i < nk
                    singles(pv[i][0], pv[i][1], a + i, start=not stop, stop=stop)
                    if stop:
                        evac(ps[i][0], ot, 2 * i)
                        evac(ps[i][1], ot, 2 * i + 1)
                if last_group:
                    # iters 0..nk-1 are fully evacuated now; ship them
                    # before the final iters' evacs so the post-compute
                    # DMA tail only carries the last iteration
                    nc.scalar.dma_start(
                        out=o_d[:, a * ow : (a + nk) * ow],
                        in_=ot[:, 0 : nk * ow],
                    )
                for i in range(nk, n_in_g):
                    k128s(pv[i][0], pv[i][1], a + i, start=False, stop=True)
                    final = last_group and i == n_in_g - 1
                    evac(ps[i][0], ot, 2 * i)
                    evac(ps[i][1], ot, 2 * i + 1, engine="vector" if final else "scalar")
                    if last_group and not final:
                        # drain the tail in per-iteration transfers so the
                        # final post-compute DMA is as small as possible
                        nc.scalar.dma_start(
                            out=o_d[:, (a + i) * ow : (a + i + 1) * ow],
                            in_=ot[:, 2 * i * B * WO : (2 * i + 2) * B * WO],
                        )
                    elif final:
                        # split the very last transfer across two trigger
                        # queues so both halves drain concurrently
                        half = B * WO
                        nc.scalar.dma_start(
                            out=o_d[:, (a + i) * ow : (a + i) * ow + half],
                            in_=ot[:, 2 * i * B * WO : (2 * i + 1) * B * WO],
                        )
                        nc.sync.dma_start(
                            out=o_d[:, (a + i) * ow + half : (a + i + 1) * ow],
                            in_=ot[:, (2 * i + 1) * B * WO : (2 * i + 2) * B * WO],
                        )
                # Scalar-engine HWDGE queue: keeps output-DMA triggers
                # (which wait on ACT results) off the input DMA paths.
                if not last_group:
                    nc.scalar.dma_start(
                        out=o_d[:, a * ow : (a + n_in_g) * ow], in_=ot[:]
                    )
                a += n_in_g
    nc.compile()
    return nc


def _prep_weights(kernels, biases):
    k = np.asarray(kernels, np.float32)  # (3,3,64,128) HWIO
    ws = []
    for kw in range(KW):  # [k0;k1] pairs (even rows, kh=0/1)
        ws.append(np.concatenate([k[0, kw], k[1, kw]], axis=0))
    for kw in range(KW):  # [k1;k2] pairs (odd rows, kh=1/2)
        ws.append(np.concatenate([k[1, kw], k[2, kw]], axis=0))
    for kw in range(KW):  # [k2;k0]: k2 low half (even kh=2), k0 high (odd kh=0)
        ws.append(np.concatenate([k[2, kw], k[0, kw]], axis=0))
    wdev = np.stack(ws, axis=1).reshape(128, 9 * F).astype(np.float16)
    # fp32 bias bits carried as two fp16 columns (device bitcasts back)
    bdev = np.asarray(biases, np.float32).reshape(128, 1).view(np.float16)
    return np.ascontiguousarray(np.concatenate([wdev, bdev], axis=1))


def kernel(**inputs):
    global _NC_CACHE, LAST_RESULT
    x = np.asarray(inputs["x"], np.float32).astype(np.float16)
    wdev = _prep_weights(inputs["kernels"], inputs["biases"])

    if _NC_CACHE is None:
        _NC_CACHE = _build_bass()
    nc = _NC_CACHE

    in_maps = []
    for i in range(N_CORES):
        xc = x[i * B : (i + 1) * B]  # [4,112,112,64]
        # [b, rp, par, w, c] -> [par, c, rp, b, w]; partition p = par*64 + c
        xp = xc.reshape(B, NRP, 2, W, C).transpose(2, 4, 1, 0, 3)
        in_maps.append(
            {"x": np.ascontiguousarray(xp).reshape(128, X_ELEMS), "w": wdev}
        )

    LAST_RESULT = run_bass_kernel_spmd(
        nc, in_maps, core_ids=list(range(N_CORES)), trace=_TRACE
    )

    outs = []
    for res in LAST_RESULT.results:
        o = res["o"].astype(np.float32).reshape(F, A, 2, B, WO).transpose(3, 1, 2, 4, 0)
        outs.append(o.reshape(B, HO, WO, F))
    return np.ascontiguousarray(np.concatenate(outs, axis=0))


# revision 12
# speedup vs baseline: 1.0592x; 1.0592x over previous
"""Trainium2 Bass kernel: 3x3 stride-1 VALID conv (NHWC, HWIO) + bias + ReLU.

Problem shapes:
  x       (32, 112, 112, 64)  f32
  kernels (3, 3, 64, 128)     f32
  biases  (128,)              f32
  out     (32, 110, 110, 128) f32

Strategy:
  * Data-parallel: 4 images per core across 8 NeuronCores (no collectives).
  * Host pre-packs x into a channel/row-parity-major layout
      X[p, rp, b, w]  with p = (h%2)*64 + c,  rp = h//2
    so adjacent image rows sit on opposite halves of the 128 SBUF
    partitions. A 3x3x64 conv then becomes 6 PSUM-accumulated matmuls
    per output row (3 kw shifts x {one K=128 row-pair matmul + one K=64
    single-row matmul}), batching all 4 images into N=440 moving columns.
  * The six K=64 "leftover" matmuls per row pair land on opposite PE
    row-halves (even rows use partitions 0-63, odd rows 64-127), so the
    hardware runs them as three concurrent row-tiled pairs -> 9 effective
    PE slots per row pair = the MAC-floor schedule (4.5 slots/row).
  * Switching the PE between 128-row and 64-row tiling modes drains the
    array (~95ns). a-iterations are processed in pairs ordered
    [A K128s][A singles][B singles][B K128s] so the drain is paid twice
    per TWO row pairs instead of twice per one.
  * Warm-up matmuls are emitted before any input-DMA trigger so the PE
    HAM clock gate (cold = 1.2GHz) releases while input DMAs are still
    in flight; input chunk triggers alternate sync/gpsimd queues so the
    first chunks land ~2us earlier than a single serial trigger queue.
  * fp32r/fp16 matmul dtype: fp16 streams 1 col/cycle with fp32 PSUM
    accumulation and ~3e-4 conv error (vs 2e-2 budget).
  * ScalarE fuses bias+ReLU on the PSUM->SBUF evacuation, writing fp16
    to halve output DMA bytes; host restores NHWC f32.
"""

import numpy as np

import concourse.bass as bass
import concourse.mybir as mybir
from concourse import bacc
from concourse.bass_utils import run_bass_kernel_spmd
from concourse.tile import TileContext

N_CORES = 8
B = 4  # images per core
H = W = 112
C = 64
F = 128
KH = KW = 3
HO = WO = 110
NRP = H // 2  # 56 row pairs per image
A = HO // 2  # 55 output row-parity iterations

F32 = mybir.dt.float32
F16 = mybir.dt.float16
MM_DTYPE = F16
OUT_DTYPE = F16

X_ELEMS = NRP * B * W  # per-partition input elements (25088)
O_ELEMS = A * 2 * B * WO  # per-partition output elements (48400)

_TRACE = False
LAST_RESULT = None
_NC_CACHE = None


def _build_bass():
    nc = bacc.Bacc("TRN2", target_bir_lowering=False, debug=False)
    x_d = nc.dram_tensor("x", [128, X_ELEMS], MM_DTYPE, kind="ExternalInput")
    # weights (9 stacked [128,128] lhsT tiles) + fp32 bias packed as the
    # last two fp16 columns (bitcast back to f32 on device)
    w_d = nc.dram_tensor("w", [128, 9 * F + 2], MM_DTYPE, kind="ExternalInput")
    o_d = nc.dram_tensor("o", [128, O_ELEMS], OUT_DTYPE, kind="ExternalOutput")

    rpw = B * W  # elems per rowpair per partition (448)
    ow = 2 * B * WO  # output elems per a-iteration (880)

    with TileContext(nc) as tc:
        with (
            tc.tile_pool(name="xres", bufs=1) as xpool,
            tc.tile_pool(name="wpool", bufs=1) as wpool,
            tc.tile_pool(name="psum", bufs=8, space="PSUM") as ppool,
            tc.tile_pool(name="opool", bufs=4) as opool,
        ):
            # PE pre-warm FIRST: dummy matmuls on a vector-memset buffer
            # while weights/input DMAs are still being triggered. Releases
            # the PE_HAM clock gate (cold = 1.2GHz) before real work
            # arrives; the scratch PSUM bank is never read. ~11 cold
            # N=440 matmuls x 367ns spans the gap until chunk data lands.
            warm = wpool.tile([128, 440], MM_DTYPE)
            nc.vector.memset(warm[:], 0.0)
            wps = ppool.tile([128, B * WO], F32, tag="ps")
            NWARM = 7
            for j in range(NWARM):
                nc.tensor.matmul(
                    wps[:], warm[:, 0:128], warm[:], start=(j == 0), stop=(j == NWARM - 1)
                )

            wt = wpool.tile([128, 9 * F + 2], MM_DTYPE)
            nc.scalar.dma_start(out=wt[:], in_=w_d[:])
            bt = wt[:, 9 * F : 9 * F + 2].bitcast(F32)

            # Fast-start chunk schedule: small chunks first so the first
            # matmul group can begin ASAP, larger chunks once compute is
            # the slower consumer. Triggers alternate between the sync
            # and gpsimd queues: two engines issue ~680ns DMA triggers in
            # parallel, and neither shares DMAHW lanes with the scalar
            # output-DMA path (head-of-line blocking).
            chunk_rps = [1, 1, 2, 4] + [8] * 6
            assert sum(chunk_rps) == NRP
            trig = [nc.sync, nc.gpsimd]
            rp2view = []  # rowpair -> (view, local index)
            for ch, nrp_ch in enumerate(chunk_rps):
                cht = xpool.tile([128, nrp_ch * rpw], MM_DTYPE, tag=f"xch{ch}")
                s = len(rp2view) * rpw
                trig[ch % 2].dma_start(out=cht[:], in_=x_d[:, s : s + nrp_ch * rpw])
                v = cht[:].rearrange("p (rp b w) -> p rp b w", rp=nrp_ch, b=B, w=W)
                for r in range(nrp_ch):
                    rp2view.append((v, r))

            def xs(lo, hi, rp, kw):
                v, r = rp2view[rp]
                return v[lo:hi, r, :, kw : kw + WO]

            wv = wt[:, 0 : 9 * F].rearrange("p (i f) -> p i f", i=9, f=F)

            def k128s(ps_par0, ps_par1, a, start, stop):
                # six full-array matmuls of one a-iteration:
                # out row 2a: kh=0,1 -> rows 2a,2a+1 (rowpair a, K=128)
                # out row 2a+1: kh=1,2 -> rows 2a+2,2a+3 (rowpair a+1, K=128)
                for kw in range(KW):
                    nc.tensor.matmul(
                        ps_par0, wv[:, kw, :], xs(0, 128, a, kw),
                        start=(start and kw == 0), stop=(stop and kw == KW - 1),
                    )
                for kw in range(KW):
                    nc.tensor.matmul(
                        ps_par1, wv[:, 3 + kw, :], xs(0, 128, a + 1, kw),
                        start=(start and kw == 0), stop=(stop and kw == KW - 1),
                    )

            def singles(ps_par0, ps_par1, a, start, stop):
                # six K=64 matmuls on opposite PE row-halves, emitted
                # interleaved so the hardware runs them as three
                # concurrent row-tiled pairs:
                # out row 2a: kh=2 -> row 2a+2 (low half of rowpair a+1)
                # out row 2a+1: kh=0 -> row 2a+1 (high half of rowpair a)
                for kw in range(KW):
                    nc.tensor.matmul(
                        ps_par0, wv[0:64, 6 + kw, :], xs(0, 64, a + 1, kw),
                        start=(start and kw == 0), stop=(stop and kw == KW - 1),
                    )
                    nc.tensor.matmul(
                        ps_par1, wv[64:128, 6 + kw, :], xs(64, 128, a, kw),
                        start=(start and kw == 0), stop=(stop and kw == KW - 1),
                    )

            def evac(ps, ot, slot, engine="scalar"):
                dst = ot[:, slot * B * WO : (slot + 1) * B * WO]
                if engine == "scalar":
                    nc.scalar.activation(
                        out=dst,
                        in_=ps[:],
                        func=mybir.ActivationFunctionType.Relu,
                        bias=bt,
                    )
                else:
                    # DVE bias+relu: out = max(psum + bias, 0)
                    nc.vector.tensor_scalar(
                        out=dst,
                        in0=ps[:],
                        scalar1=bt,
                        scalar2=0.0,
                        op0=mybir.AluOpType.add,
                        op1=mybir.AluOpType.max,
                    )

            # Process a-iterations in groups of 4 ordered
            # [A K][B K][A s][B s][C s][D s][C K][D K]: the PE pays the
            # two 128row<->64row tiling-mode drains (~95ns each) per FOUR
            # row pairs. A group's 8 PSUM banks exactly fill the ring, so
            # bank reuse happens across groups (~7us apart) and never
            # stalls the PE (8-iter groups measurably stall on both the
            # mid-group bank wrap and early input-chunk demand).
            a = 0
            while a < A:
                n_in_g = min(4, A - a)
                last_group = a + n_in_g >= A
                ot = opool.tile([128, n_in_g * ow], OUT_DTYPE, tag="ot")
                ps = []
                pv = []
                for i in range(n_in_g):
                    p0 = ppool.tile([128, B * WO], F32, tag="ps", name=f"p{i}0")
                    p1 = ppool.tile([128, B * WO], F32, tag="ps", name=f"p{i}1")
                    ps.append((p0, p1))
                    pv.append((
                        p0[:].rearrange("p (b w) -> p b w", b=B),
                        p1[:].rearrange("p (b w) -> p b w", b=B),
                    ))
                nk = (n_in_g + 1) // 2  # iters whose K128s lead
                for i in range(nk):
                    k128s(pv[i][0], pv[i][1], a + i, start=True, stop=False)
                for i in range(n_in_g):
                    stop = i < nk
                    singles(pv[i][0], pv[i][1], a + i, start=not stop, stop=stop)
                    if stop:
                        evac(ps[i][0], ot, 2 * i)
                        evac(ps[i][1], ot, 2 * i + 1)
                if last_group:
                    # iters 0..nk-1 are fully evacuated now; ship them
                    # before the final iters' evacs so the post-compute
                    # DMA tail only carries the last iteration
                    nc.scalar.dma_start(
                        out=o_d[:, a * ow : (a + nk) * ow],
                        in_=ot[:, 0 : nk * ow],
                    )
                for i in range(nk, n_in_g):
                    k128s(pv[i][0], pv[i][1], a + i, start=False, stop=True)
                    final = last_group and i == n_in_g - 1
                    evac(ps[i][0], ot, 2 * i)
                    evac(ps[i][1], ot, 2 * i + 1, engine="vector" if final else "scalar")
                    if last_group and not final:
                        # drain the tail in per-iteration transfers so the
                        # final post-compute DMA is as small as possible
                        nc.scalar.dma_start(
                            out=o_d[:, (a + i) * ow : (a + i + 1) * ow],
                            in_=ot[:, 2 * i * B * WO : (2 * i + 2) * B * WO],
                        )
                    elif final:
                        # split the very last transfer across two trigger
                        # queues so both halves drain concurrently
                        half = B * WO
                        nc.scalar.dma_start(
                            out=o_d[:, (a + i) * ow : (a + i) * ow + half],
                            in_=ot[:, 2 * i * B * WO : (2 * i + 1) * B * WO],
                        )
                        nc.sync.dma_start(
                            out=o_d[:, (a + i) * ow + half : (a + i + 1) * ow],
                            in_=ot[:, (2 * i + 1) * B * WO : (2 * i + 2) * B * WO],
                        )
                # Scalar-engine HWDGE queue: keeps output-DMA triggers
                # (which wait on ACT results) off the input DMA paths.
                if not last_group:
                    nc.scalar.dma_start(
                        out=o_d[:, a * ow : (a + n_in_g) * ow], in_=ot[:]
                    )
                a += n_in_g
    nc.compile()
    return nc


def _prep_weights(kernels, biases):
    k = np.asarray(kernels, np.float32)  # (3,3,64,128) HWIO
    ws = []
    for kw in range(KW):  # [k0;k1] pairs (even rows, kh=0/1)
        ws.append(np.concatenate([k[0, kw], k[1, kw]], axis=0))
    for kw in range(KW):  # [k1;k2] pairs (odd rows, kh=1/2)
        ws.append(np.concatenate([k[1, kw], k[2, kw]], axis=0))
    for kw in range(KW):  # [k2;k0]: k2 low half (even kh=2), k0 high (odd kh=0)
        ws.append(np.concatenate([k[2, kw], k[0, kw]], axis=0))
    wdev = np.stack(ws, axis=1).reshape(128, 9 * F).astype(np.float16)
    # fp32 bias bits carried as two fp16 columns (device bitcasts back)
    bdev = np.asarray(biases, np.float32).reshape(128, 1).view(np.float16)
    return np.ascontiguousarray(np.concatenate([wdev, bdev], axis=1))


def kernel(**inputs):
    global _NC_CACHE, LAST_RESULT
    x = np.asarray(inputs["x"], np.float32).astype(np.float16)
    wdev = _prep_weights(inputs["kernels"], inputs["biases"])

    if _NC_CACHE is None:
        _NC_CACHE = _build_bass()
    nc = _NC_CACHE

    in_maps = []
    for i in range(N_CORES):
        xc = x[i * B : (i + 1) * B]  # [4,112,112,64]
        # [b, rp, par, w, c] -> [par, c, rp, b, w]; partition p = par*64 + c
        xp = xc.reshape(B, NRP, 2, W, C).transpose(2, 4, 1, 0, 3)
        in_maps.append(
            {"x": np.ascontiguousarray(xp).reshape(128, X_ELEMS), "w": wdev}
        )

    LAST_RESULT = run_bass_kernel_spmd(
        nc, in_maps, core_ids=list(range(N_CORES)), trace=_TRACE
    )

    outs = []
    for res in LAST_RESULT.results:
        o = res["o"].astype(np.float32).reshape(F, A, 2, B, WO).transpose(3, 1, 2, 4, 0)
        outs.append(o.reshape(B, HO, WO, F))
    return np.ascontiguousarray(np.concatenate(outs, axis=0))
